# revision 36
# baseline (speedup 1.0000x reference)
"""Trainium2 Bass kernel for EruSelfAttentionModel.

Math (reference, simplified):
  e  = emb_table[x]                                  # [B,S,E] gather
  h  = LayerNorm(e) * gamma + beta                   # over E
  q  = einsum('hae,bse->bhsa', Wq, h); k likewise    # A=64 per head
  v  = einsum('hve,bse->bhsv', Wv, h)                # v-dim = E
  scores = q @ k^T / sqrt(E)
  sn = (scores - min) / (max - min)  (rowwise)
  softmax_sel = 1 - max(sn) == 0 exactly  =>  weights = sigmoid(10*sn - 5)
  out = weights @ v                                  # [B,H,S,E]

Key identities used:
  - sn is invariant to positive rescaling of scores => the 1/sqrt(E) scale
    can be dropped entirely.
  - weights = sigmoid(alpha * scores + beta_row) with per-row
    alpha = 10/(mx-mn), beta_row = -10*mn/(mx-mn) - 5  => single fused
    ScalarE activation pass (per-partition scale/bias APs).

Sharding: data-parallel over batch; core b computes batch b fully.

Schedule (vs the original baseline):
  - Startup pipelined: per-chunk gather -> LN -> PE transpose; Q/K
    projection slices issued as their token half is transposed, with
    head-pair slice 0 first so attention units can start early.  The
    4MB wvt streams in 8 per-head slices on the ACT HWDGE queue, gated
    behind the gather traffic via a WAW dirty-corner dependency.
  - Attention units emit out-stage FIRST, then vhat pieces (spread to
    keep TensorE dense), then scores; the two heads' 64-row score
    matmuls run concurrently via PE row tiling.
  - scores copy (wraw) in bf16; row-max fused into the PSUM->SBUF copy
    (DVE accum); out-stage stores issued from the ACT sequencer (Sync
    is saturated by W^T transpose descriptor generation).
  - PSUM: scores ring 2x[P,S] (4 banks), vhat/proj ring 2x[P,512]
    (2 banks), out/transpose ring 2x[P,512] (2 banks).
"""

import os
import sys

sys.path.insert(0, "/opt/trn_rl_repo")

import numpy as np
import ml_dtypes

import concourse.bass as bass
import concourse.bacc as bacc
import concourse.tile as tile
from concourse import mybir
from concourse.bass_utils import run_bass_kernel_spmd

BF16 = ml_dtypes.bfloat16

VOCAB, E, A, H = 32000, 512, 64, 8
B, S = 8, 1024
P = 128                 # partitions
NCH = S // P            # 8 token chunks
EC = E // P             # 4 embedding chunks
LN_EPS = 1e-5

F32 = mybir.dt.float32
BF = mybir.dt.bfloat16
I16 = mybir.dt.int16

_BUILD_CACHE = {}
LAST_RESULTS = None     # test.py reads exec_time_ns from here


def build_nc(use_beta: bool):
    if (use_beta,) in _BUILD_CACHE:
        return _BUILD_CACHE[(use_beta,)]

    nc = bacc.Bacc("TRN2", target_bir_lowering=False, num_devices=8)

    idx_d = nc.declare_dram_parameter("idx", [P, S // 16], I16, isOutput=False)
    emb_d = nc.declare_dram_parameter("emb", [VOCAB, E], F32, isOutput=False)
    wqt_d = nc.declare_dram_parameter("wqt", [E, H * A], BF, isOutput=False)
    wkt_d = nc.declare_dram_parameter("wkt", [E, H * A], BF, isOutput=False)
    wvt_d = nc.declare_dram_parameter("wvt", [E, H * E], BF, isOutput=False)
    idn_d = nc.declare_dram_parameter("idn", [P, P], BF, isOutput=False)
    if use_beta:
        qb_d = nc.declare_dram_parameter("qb", [P, 4], F32, isOutput=False)
        kb_d = nc.declare_dram_parameter("kb", [P, 4], F32, isOutput=False)
        vb_d = nc.declare_dram_parameter("vb", [1, H * E], F32, isOutput=False)
    out_d = nc.declare_dram_parameter("out", [H, S, E], BF, isOutput=True)

    with tile.TileContext(nc) as tc:
        with tc.tile_pool(name="consts", bufs=1) as consts:
            # small/early DMAs first so the gather chain and Q/K proj are
            # not stuck behind the 4MB wvt transfer.
            idx_sb = consts.tile([P, S // 16], I16)
            nc.sync.dma_start(idx_sb[:], idx_d[:])
            wqt_sb = consts.tile([P, EC, H * A], BF)
            nc.sync.dma_start(
                wqt_sb[:], wqt_d.ap().rearrange("(ec p) j -> p ec j", p=P)
            )
            wkt_sb = consts.tile([P, EC, H * A], BF)
            nc.sync.dma_start(
                wkt_sb[:], wkt_d.ap().rearrange("(ec p) j -> p ec j", p=P)
            )
            idn_sb = consts.tile([P, P], BF)
            nc.sync.dma_start(idn_sb[:], idn_d[:])
            # 4MB wvt loads as 8 per-head column slices on the ACT HWDGE
            # queue, issued lazily inside the LN chunk loop, so the Sync
            # queue drains early and the embedding gathers get DMA-ring
            # credit; vhat block vp only needs slice vp.
            wvt_sb = consts.tile([P, EC, H * E], BF)
            wvt_src = wvt_d.ap().rearrange("(ec p) j -> p ec j", p=P)
            eps_sb = consts.tile([P, 1], F32)
            nc.vector.memset(eps_sb[:], LN_EPS)
            ten_sb = consts.tile([P, 2], F32)
            nc.vector.memset(ten_sb[:], 10.0)

            if use_beta:
                qb_sb = consts.tile([P, 4], F32)
                nc.sync.dma_start(qb_sb[:], qb_d[:])
                kb_sb = consts.tile([P, 4], F32)
                nc.sync.dma_start(kb_sb[:], kb_d[:])
                vb_sb = consts.tile([P, H * E], F32)
                vb_bcast = bass.AP(
                    tensor=vb_d, offset=0, ap=[[0, P], [1, H * E]]
                )
                nc.sync.dma_start(vb_sb[:], vb_bcast)

            # persistent activations
            hT_sb = consts.tile([P, EC, S], BF)       # hT[e%128, e//128, s]
            qT_sb = consts.tile([P, EC, S], BF)       # qT[ha%128, ha//128, s]
            kT_sb = consts.tile([P, EC, S], BF)
            vh_sb = consts.tile([P, NCH, H * E], BF)  # vh[p, c, v] = V[8p+c, v]

            # All PSUM pools share one scope (8 banks total):
            #   sc 2x[P,S] = 4 banks; pv 2x[P,512] = 2 (vhat + Q/K proj
            #   accumulators); po 2x[P,512] = 2 (out stages + LN transposes).
            with (
                tc.tile_pool(name="sc_psum", bufs=2, space="PSUM") as sc_psum,
                tc.tile_pool(name="vh_psum", bufs=2, space="PSUM") as vh_psum,
                tc.tile_pool(name="out_psum", bufs=2, space="PSUM") as out_psum,
                tc.tile_pool(name="e_pool", bufs=6) as e_pool,
                tc.tile_pool(name="st_pool", bufs=8) as st_pool,
                tc.tile_pool(name="h_pool", bufs=3) as h_pool,
                tc.tile_pool(name="sstat", bufs=8) as sstat,
                tc.tile_pool(name="w_pool", bufs=4) as w_pool,
                tc.tile_pool(name="wraw_pool", bufs=4) as wraw_pool,
                tc.tile_pool(name="wt_pool", bufs=12) as wt_pool,
                tc.tile_pool(name="ob_pool", bufs=4) as ob_pool,
            ):
                mv = st_pool.tile([P, NCH, 2], F32, tag="mv")

                def proj_slice(nn, sl):
                    # q/k projections for token half nn, (h,a) slice sl
                    for name, w_sb, t_sb in (
                        ("q", wqt_sb, qT_sb),
                        ("k", wkt_sb, kT_sb),
                    ):
                        pq = vh_psum.tile([P, 512], F32, tag="pv")
                        for ec in range(EC):
                            nc.tensor.matmul(
                                pq[:],
                                w_sb[:, ec, sl * P : (sl + 1) * P],
                                hT_sb[:, ec, nn * 512 : (nn + 1) * 512],
                                start=(ec == 0), stop=(ec == EC - 1),
                            )
                        if use_beta:
                            bb = qb_sb if name == "q" else kb_sb
                            nc.vector.tensor_scalar_add(
                                out=pq[:], in0=pq[:],
                                scalar1=bb[:, sl : sl + 1],
                            )
                        nc.scalar.copy(
                            t_sb[:, sl, nn * 512 : (nn + 1) * 512], pq[:]
                        )

                # ---- phase A+B: gather + LN + transpose + Q/K proj,
                #      pipelined per token chunk ----
                for c in range(NCH):
                    e_t = e_pool.tile([P, 1, E], F32, tag="e")
                    nc.gpsimd.dma_gather(
                        e_t[:], emb_d.ap(), idx_sb[:, 8 * c : 8 * (c + 1)],
                        P, P, E,
                    )
                    stt = st_pool.tile([P, 6], F32, tag="bn")
                    nc.vector.bn_stats(stt[:], e_t[:, 0, :])
                    nc.vector.bn_aggr(mv[:, c, :], stt[:])
                    # rstd = 1/sqrt(var+eps): ScalarE sqrt + DVE reciprocal
                    nc.scalar.activation(
                        out=mv[:, c, 1:2], in_=mv[:, c, 1:2],
                        func=mybir.ActivationFunctionType.Sqrt,
                        bias=eps_sb[:, 0:1], scale=1.0,
                    )
                    # high_priority: keep each chunk's LN tail ahead of later
                    # chunks' stats in the static DVE stream, so hT chunks
                    # complete as the gathers land instead of bunching late
                    with tc.high_priority():
                        nc.vector.reciprocal(mv[:, c, 1:2], mv[:, c, 1:2])
                        h_t = h_pool.tile([P, E], BF, tag="h")
                        nc.vector.tensor_scalar(
                            out=h_t[:], in0=e_t[:, 0, :],
                            scalar1=mv[:, c, 0:1], scalar2=mv[:, c, 1:2],
                            op0=mybir.AluOpType.subtract, op1=mybir.AluOpType.mult,
                        )
                        for ec in range(EC):
                            pt = out_psum.tile([P, P], BF, tag="po")
                            nc.tensor.transpose(
                                pt[:], h_t[:, ec * P : (ec + 1) * P], idn_sb[:]
                            )
                            nc.vector.tensor_copy(
                                hT_sb[:, ec, c * P : (c + 1) * P], pt[:]
                            )
                    if c == 3:
                        for sl in range(4):
                            proj_slice(0, sl)
                proj_slice(1, 0)

                # wvt slice DMAs, gated behind the gather traffic: a tiny
                # Pool-engine copy (dep: chunk-5 LN stats) dirties each
                # slice's corner, so the scheduler cannot hoist the 4MB of
                # wvt transfers ahead of the embedding gathers.
                for c in range(NCH):
                    nc.gpsimd.tensor_copy(
                        wvt_sb[0:1, 0:1, c * 512 : c * 512 + 1],
                        mv[0:1, 5:6, 1:2],
                    )
                    nc.scalar.dma_start(
                        wvt_sb[:, :, c * 512 : (c + 1) * 512],
                        wvt_src[:, :, c * 512 : (c + 1) * 512],
                    )

                # ---------------- phase C: attention ----------------
                pending = []

                def score_unit(hp, i):
                    ps = []
                    for sub in range(2):  # head within pair
                        p0 = sub * 64
                        psc = sc_psum.tile([P, S], F32, tag="sc")
                        for nn in range(2):
                            nc.tensor.matmul(
                                psc[:, nn * 512 : (nn + 1) * 512],
                                qT_sb[p0 : p0 + 64, hp, i * P : (i + 1) * P],
                                kT_sb[p0 : p0 + 64, hp, nn * 512 : (nn + 1) * 512],
                                start=True, stop=True,
                            )
                        ps.append(psc)
                    # rowwise min/max -> alpha/beta for fused sigmoid
                    # layout: [maxA, maxB, minA, minB, betaA, betaB, alpA, alpB]
                    st = sstat.tile([P, 8], F32, tag="st")
                    wraws = []
                    for sub in range(2):
                        # bf16: halves the GpSimd min level and the ACT
                        # sigmoid read; costs ~0.1% extra rel err (simulated)
                        wraw = wraw_pool.tile([P, S], BF, tag="wr")
                        # fused PSUM->SBUF copy + row-max (accum) on DVE
                        nc.vector.tensor_scalar(
                            out=wraw[:], in0=ps[sub][:],
                            scalar1=-3.0e38, scalar2=None,
                            op0=mybir.AluOpType.max, op1=mybir.AluOpType.max,
                            accum_out=st[:, sub : sub + 1],
                        )
                        wraws.append(wraw)
                        nc.vector.tensor_reduce(
                            st[:, 2 + sub : 3 + sub], wraw[:],
                            axis=mybir.AxisListType.X, op=mybir.AluOpType.min,
                        )
                    mx = st[:, 0:2]
                    mn = st[:, 2:4]
                    rng = st[:, 4:6]
                    alp = st[:, 6:8]
                    nc.vector.tensor_sub(rng, mx, mn)
                    nc.vector.reciprocal(rng, rng)
                    nc.vector.tensor_scalar_mul(alp, rng, 10.0)
                    # beta = -(mn*alpha) - 5  (reuse rng slot)
                    nc.vector.tensor_mul(rng, mn, alp)
                    nc.vector.tensor_scalar(
                        out=rng, in0=rng, scalar1=-1.0, scalar2=-5.0,
                        op0=mybir.AluOpType.mult, op1=mybir.AluOpType.add,
                    )
                    wts = []
                    for sub in range(2):
                        w_t = w_pool.tile([P, S], BF, tag="w")
                        nc.scalar.activation(
                            out=w_t[:], in_=wraws[sub][:],
                            func=mybir.ActivationFunctionType.Sigmoid,
                            bias=st[:, 4 + sub : 5 + sub],
                            scale=st[:, 6 + sub : 7 + sub],
                        )
                        wt_t = wt_pool.tile([P, NCH, P], BF, tag="wt")
                        nc.sync.dma_start_transpose(wt_t[:], w_t[:])
                        wts.append(wt_t)
                    pending.append((hp, i, wts))

                units = [(hp, i) for hp in range(4) for i in range(NCH)]
                # vhat work list: (vp, c) pieces.  Block vp (= V columns of
                # head vp) must be done before head pair vp//2's out-stages
                # start at unit 8*(vp//2)+PEND.  Spread evenly so PE stays
                # the binding engine in every unit, with the wvt DMA slices
                # streaming in behind the gather traffic.
                PEND = 5
                pieces = [(vp, c) for vp in range(8) for c in range(NCH)]
                # blocks 0-1 (16 pieces) across units 0-4; blocks 2-7 (48)
                # at 2/unit over units 5-28
                vhat_sched = {u: pieces[3 * u : 3 * u + 3] for u in range(4)}
                vhat_sched[4] = pieces[12:16]
                late = pieces[16:]
                for k in range(24):
                    vhat_sched[5 + k] = late[2 * k : 2 * k + 2]

                def vhat_piece(vp, c, eng=0):
                    # V columns of head vp, token chunk c.
                    lo = vp * 512
                    pv = vh_psum.tile([P, 512], F32, tag="pv")
                    for ec in range(EC):
                        nc.tensor.matmul(
                            pv[:],
                            hT_sb[:, ec, c * P : (c + 1) * P],
                            wvt_sb[:, ec, lo : lo + 512],
                            start=(ec == 0), stop=(ec == EC - 1),
                        )
                    # alternate the psum->SBUF copy between ACT and DVE to
                    # keep either engine from saturating
                    if eng == 0:
                        nc.scalar.copy(vh_sb[:, c, lo : lo + 512], pv[:])
                    else:
                        nc.vector.tensor_copy(vh_sb[:, c, lo : lo + 512], pv[:])

                def out_stage(args):
                    hp_, i_, wts = args
                    for sub in range(2):
                        h_idx = 2 * hp_ + sub
                        wt_t = wts[sub]
                        po = out_psum.tile([P, E], F32, tag="po")
                        for cc in range(NCH):
                            nc.tensor.matmul(
                                po[:],
                                wt_t[:, cc, :],
                                vh_sb[:, cc, h_idx * E : (h_idx + 1) * E],
                                start=(cc == 0), stop=(cc == NCH - 1),
                            )
                        ob = ob_pool.tile([P, E], BF, tag="ob")
                        nc.scalar.copy(ob[:], po[:])
                        # issue the store via SWDGE on the idle Pool engine:
                        # Sync is saturated by W^T transpose desc-gen and ACT
                        # by sigmoids/copies
                        nc.gpsimd.dma_start(
                            out_d[h_idx, i_ * P : (i_ + 1) * P, :], ob[:]
                        )

                for u, (hp, i) in enumerate(units):
                    if u in (2, 4, 6):
                        proj_slice(1, u // 2)
                    if u >= PEND:
                        out_stage(pending.pop(0))
                    for pi, (vp, c) in enumerate(vhat_sched.get(u, ())):
                        vhat_piece(vp, c, eng=pi % 2)
                    score_unit(hp, i)
                for pp_ in pending:
                    out_stage(pp_)

    nc.compile()
    _BUILD_CACHE[(use_beta,)] = nc
    return nc


def _prep_inputs(x, emb_table, gamma, beta, Wq, Wk, Wv, use_beta):
    x = np.asarray(x)
    gamma = np.asarray(gamma, dtype=np.float32)
    beta = np.asarray(beta, dtype=np.float32)
    Wq = np.asarray(Wq, dtype=np.float32)
    Wk = np.asarray(Wk, dtype=np.float32)
    Wv = np.asarray(Wv, dtype=np.float32)
    emb = np.ascontiguousarray(np.asarray(emb_table, dtype=np.float32))

    # W'[h,a,e] = W[h,a,e] * gamma[e]; layouts [e, h*ad+a]
    wqt = np.ascontiguousarray(
        (Wq * gamma[None, None, :]).reshape(H * A, E).T.astype(BF16)
    )
    wkt = np.ascontiguousarray(
        (Wk * gamma[None, None, :]).reshape(H * A, E).T.astype(BF16)
    )
    wvt = np.ascontiguousarray(
        (Wv * gamma[None, None, :]).reshape(H * E, E).T.astype(BF16)
    )
    idn = np.eye(P, dtype=np.float32).astype(BF16)

    consts = dict(emb=emb, wqt=wqt, wkt=wkt, wvt=wvt, idn=idn)
    if use_beta:
        qb = (Wq.reshape(H * A, E) @ beta).astype(np.float32)   # [512]
        kb = (Wk.reshape(H * A, E) @ beta).astype(np.float32)
        vb = (Wv.reshape(H * E, E) @ beta).astype(np.float32)   # [4096]
        consts["qb"] = np.ascontiguousarray(qb.reshape(4, P).T)
        consts["kb"] = np.ascontiguousarray(kb.reshape(4, P).T)
        consts["vb"] = vb.reshape(1, H * E)

    in_maps = []
    for b in range(B):
        xi = x[b].astype(np.int64)
        idx16 = np.ascontiguousarray(
            xi.reshape(S // 16, 16).T.astype(np.int16)
        )  # [16, 64]; token j of chunk c sits at [j%16, 8c + j//16]
        idx_full = np.ascontiguousarray(np.tile(idx16, (8, 1)))  # [128, 64]
        in_maps.append(dict(idx=idx_full, **consts))
    return in_maps


def kernel(x, emb_table, gamma, beta, Wq, Wk, Wv):
    global LAST_RESULTS
    beta_arr = np.asarray(beta, dtype=np.float32)
    use_beta = bool(np.any(beta_arr != 0.0))

    nc = build_nc(use_beta)
    in_maps = _prep_inputs(x, emb_table, gamma, beta, Wq, Wk, Wv, use_beta)

    trace = os.environ.get("KERNEL_TRACE", "0") == "1"
    res = run_bass_kernel_spmd(
        nc, in_maps, core_ids=list(range(B)), trace=trace
    )
    LAST_RESULTS = res

    out = np.stack([np.asarray(res.results[b]["out"]) for b in range(B)], axis=0)
    return out.astype(np.float32)


if __name__ == "__main__":
    rng = np.random.default_rng(0)
    x = rng.integers(0, VOCAB, size=(B, S), dtype=np.int32)
    emb = rng.standard_normal((VOCAB, E), dtype=np.float32)
    gamma = np.ones(E, np.float32)
    beta = np.zeros(E, np.float32)
    Wq = rng.random((H, A, E), dtype=np.float32)
    Wk = rng.random((H, A, E), dtype=np.float32)
    Wv = rng.random((H, E, E), dtype=np.float32)
    out = kernel(x, emb, gamma, beta, Wq, Wk, Wv)
    print(out.shape, out.dtype)


# revision 38
# speedup vs baseline: 1.0905x; 1.0905x over previous
"""Trainium2 Bass kernel for EruSelfAttentionModel.

Math (reference, simplified):
  e  = emb_table[x]                                  # [B,S,E] gather
  h  = LayerNorm(e) * gamma + beta                   # over E
  q  = einsum('hae,bse->bhsa', Wq, h); k likewise    # A=64 per head
  v  = einsum('hve,bse->bhsv', Wv, h)                # v-dim = E
  scores = q @ k^T / sqrt(E)
  sn = (scores - min) / (max - min)  (rowwise)
  softmax_sel = 1 - max(sn) == 0 exactly  =>  weights = sigmoid(10*sn - 5)
  out = weights @ v                                  # [B,H,S,E]

Key identities used:
  - sn is invariant to positive rescaling of scores => the 1/sqrt(E) scale
    can be dropped entirely.
  - weights = sigmoid(alpha * scores + beta_row) with per-row
    alpha = 10/(mx-mn), beta_row = -10*mn/(mx-mn) - 5  => single fused
    ScalarE activation pass (per-partition scale/bias APs).

Sharding: data-parallel over batch; core b computes batch b fully.

Schedule (vs the original baseline):
  - Startup pipelined: per-chunk gather -> LN -> PE transpose; Q/K
    projection slices issued as their token half is transposed, with
    head-pair slice 0 first so attention units can start early.  The
    4MB wvt streams in 8 per-head slices on the ACT HWDGE queue, gated
    behind the gather traffic via a WAW dirty-corner dependency.
  - Attention units emit out-stage FIRST, then vhat pieces (spread to
    keep TensorE dense), then scores; the two heads' 64-row score
    matmuls run concurrently via PE row tiling.
  - scores copy (wraw) in bf16; row-max fused into the PSUM->SBUF copy
    (DVE accum); out-stage stores issued from the ACT sequencer (Sync
    is saturated by W^T transpose descriptor generation).
  - PSUM: scores ring 2x[P,S] (4 banks), vhat/proj ring 2x[P,512]
    (2 banks), out/transpose ring 2x[P,512] (2 banks).
"""

import os
import sys

sys.path.insert(0, "/opt/trn_rl_repo")

import numpy as np
import ml_dtypes

import concourse.bass as bass
import concourse.bacc as bacc
import concourse.tile as tile
from concourse import mybir
from concourse.bass_utils import run_bass_kernel_spmd

BF16 = ml_dtypes.bfloat16

VOCAB, E, A, H = 32000, 512, 64, 8
B, S = 8, 1024
P = 128                 # partitions
NCH = S // P            # 8 token chunks
EC = E // P             # 4 embedding chunks
LN_EPS = 1e-5

F32 = mybir.dt.float32
BF = mybir.dt.bfloat16
I16 = mybir.dt.int16

_BUILD_CACHE = {}
LAST_RESULTS = None     # test.py reads exec_time_ns from here


def build_nc(use_beta: bool):
    if (use_beta,) in _BUILD_CACHE:
        return _BUILD_CACHE[(use_beta,)]

    nc = bacc.Bacc("TRN2", target_bir_lowering=False, num_devices=8)

    idx_d = nc.declare_dram_parameter("idx", [P, S // 16], I16, isOutput=False)
    emb_d = nc.declare_dram_parameter("emb", [VOCAB, E], F32, isOutput=False)
    wqt_d = nc.declare_dram_parameter("wqt", [E, H * A], BF, isOutput=False)
    wkt_d = nc.declare_dram_parameter("wkt", [E, H * A], BF, isOutput=False)
    wvt_d = nc.declare_dram_parameter("wvt", [E, H * E], BF, isOutput=False)
    idn_d = nc.declare_dram_parameter("idn", [P, P], BF, isOutput=False)
    if use_beta:
        qb_d = nc.declare_dram_parameter("qb", [P, 4], F32, isOutput=False)
        kb_d = nc.declare_dram_parameter("kb", [P, 4], F32, isOutput=False)
        vb_d = nc.declare_dram_parameter("vb", [1, H * E], F32, isOutput=False)
    out_d = nc.declare_dram_parameter("out", [H, S, E], BF, isOutput=True)

    with tile.TileContext(nc) as tc:
        with tc.tile_pool(name="consts", bufs=1) as consts:
            # small/early DMAs first so the gather chain and Q/K proj are
            # not stuck behind the 4MB wvt transfer.
            idx_sb = consts.tile([P, S // 16], I16)
            nc.sync.dma_start(idx_sb[:], idx_d[:])
            wqt_sb = consts.tile([P, EC, H * A], BF)
            nc.sync.dma_start(
                wqt_sb[:], wqt_d.ap().rearrange("(ec p) j -> p ec j", p=P)
            )
            wkt_sb = consts.tile([P, EC, H * A], BF)
            nc.sync.dma_start(
                wkt_sb[:], wkt_d.ap().rearrange("(ec p) j -> p ec j", p=P)
            )
            idn_sb = consts.tile([P, P], BF)
            nc.sync.dma_start(idn_sb[:], idn_d[:])
            # 4MB wvt loads as 8 per-head column slices on the ACT HWDGE
            # queue, issued lazily inside the LN chunk loop, so the Sync
            # queue drains early and the embedding gathers get DMA-ring
            # credit; vhat block vp only needs slice vp.
            wvt_sb = consts.tile([P, EC, H * E], BF)
            wvt_src = wvt_d.ap().rearrange("(ec p) j -> p ec j", p=P)
            eps_sb = consts.tile([P, 1], F32)
            nc.vector.memset(eps_sb[:], LN_EPS)
            ten_sb = consts.tile([P, 2], F32)
            nc.vector.memset(ten_sb[:], 10.0)

            if use_beta:
                qb_sb = consts.tile([P, 4], F32)
                nc.sync.dma_start(qb_sb[:], qb_d[:])
                kb_sb = consts.tile([P, 4], F32)
                nc.sync.dma_start(kb_sb[:], kb_d[:])
                vb_sb = consts.tile([P, H * E], F32)
                vb_bcast = bass.AP(
                    tensor=vb_d, offset=0, ap=[[0, P], [1, H * E]]
                )
                nc.sync.dma_start(vb_sb[:], vb_bcast)

            # persistent activations
            hT_sb = consts.tile([P, EC, S], BF)       # hT[e%128, e//128, s]
            qT_sb = consts.tile([P, EC, S], BF)       # qT[ha%128, ha//128, s]
            kT_sb = consts.tile([P, EC, S], BF)
            vh_sb = consts.tile([P, NCH, H * E], BF)  # vh[p, c, v] = V[8p+c, v]

            # All PSUM pools share one scope (8 banks total):
            #   sc 2x[P,S] = 4 banks; pv 2x[P,512] = 2 (vhat + Q/K proj
            #   accumulators); po 2x[P,512] = 2 (out stages + LN transposes).
            with (
                tc.tile_pool(name="sc_psum", bufs=2, space="PSUM") as sc_psum,
                tc.tile_pool(name="vh_psum", bufs=2, space="PSUM") as vh_psum,
                tc.tile_pool(name="out_psum", bufs=2, space="PSUM") as out_psum,
                tc.tile_pool(name="e_pool", bufs=6) as e_pool,
                tc.tile_pool(name="st_pool", bufs=8) as st_pool,
                tc.tile_pool(name="h_pool", bufs=3) as h_pool,
                tc.tile_pool(name="sstat", bufs=8) as sstat,
                tc.tile_pool(name="w_pool", bufs=4) as w_pool,
                tc.tile_pool(name="wraw_pool", bufs=4) as wraw_pool,
                tc.tile_pool(name="wt_pool", bufs=12) as wt_pool,
                tc.tile_pool(name="ob_pool", bufs=3) as ob_pool,
            ):
                mv = st_pool.tile([P, NCH, 2], F32, tag="mv")

                def proj_slice(nn, sl):
                    # q/k projections for token half nn, (h,a) slice sl
                    for name, w_sb, t_sb in (
                        ("q", wqt_sb, qT_sb),
                        ("k", wkt_sb, kT_sb),
                    ):
                        pq = vh_psum.tile([P, 512], F32, tag="pv")
                        for ec in range(EC):
                            nc.tensor.matmul(
                                pq[:],
                                w_sb[:, ec, sl * P : (sl + 1) * P],
                                hT_sb[:, ec, nn * 512 : (nn + 1) * 512],
                                start=(ec == 0), stop=(ec == EC - 1),
                            )
                        if use_beta:
                            bb = qb_sb if name == "q" else kb_sb
                            nc.vector.tensor_scalar_add(
                                out=pq[:], in0=pq[:],
                                scalar1=bb[:, sl : sl + 1],
                            )
                        nc.scalar.copy(
                            t_sb[:, sl, nn * 512 : (nn + 1) * 512], pq[:]
                        )

                # ---- phase A+B: gather + LN + transpose + Q/K proj,
                #      pipelined per token chunk ----
                for c in range(NCH):
                    e_t = e_pool.tile([P, 1, E], F32, tag="e")
                    nc.gpsimd.dma_gather(
                        e_t[:], emb_d.ap(), idx_sb[:, 8 * c : 8 * (c + 1)],
                        P, P, E,
                    )
                    stt = st_pool.tile([P, 6], F32, tag="bn")
                    nc.vector.bn_stats(stt[:], e_t[:, 0, :])
                    nc.vector.bn_aggr(mv[:, c, :], stt[:])
                    # rstd = 1/sqrt(var+eps): ScalarE sqrt + DVE reciprocal
                    nc.scalar.activation(
                        out=mv[:, c, 1:2], in_=mv[:, c, 1:2],
                        func=mybir.ActivationFunctionType.Sqrt,
                        bias=eps_sb[:, 0:1], scale=1.0,
                    )
                    # high_priority: keep each chunk's LN tail ahead of later
                    # chunks' stats in the static DVE stream, so hT chunks
                    # complete as the gathers land instead of bunching late
                    with tc.high_priority():
                        nc.vector.reciprocal(mv[:, c, 1:2], mv[:, c, 1:2])
                        h_t = h_pool.tile([P, E], BF, tag="h")
                        nc.vector.tensor_scalar(
                            out=h_t[:], in0=e_t[:, 0, :],
                            scalar1=mv[:, c, 0:1], scalar2=mv[:, c, 1:2],
                            op0=mybir.AluOpType.subtract, op1=mybir.AluOpType.mult,
                        )
                        for ec in range(EC):
                            pt = out_psum.tile([P, P], BF, tag="po")
                            nc.tensor.transpose(
                                pt[:], h_t[:, ec * P : (ec + 1) * P], idn_sb[:]
                            )
                            nc.vector.tensor_copy(
                                hT_sb[:, ec, c * P : (c + 1) * P], pt[:]
                            )
                    if c == 3:
                        for sl in range(4):
                            proj_slice(0, sl)
                proj_slice(1, 0)

                # wvt slice DMAs, gated behind the gather traffic: a tiny
                # Pool-engine copy (dep: chunk-5 LN stats) dirties each
                # slice's corner, so the scheduler cannot hoist the 4MB of
                # wvt transfers ahead of the embedding gathers.
                for c in range(NCH):
                    nc.gpsimd.tensor_copy(
                        wvt_sb[0:1, 0:1, c * 512 : c * 512 + 1],
                        mv[0:1, 5:6, 1:2],
                    )
                    nc.scalar.dma_start(
                        wvt_sb[:, :, c * 512 : (c + 1) * 512],
                        wvt_src[:, :, c * 512 : (c + 1) * 512],
                    )

                # ---------------- phase C: attention ----------------
                pending = []

                def score_unit(hp, i):
                    ps = []
                    for sub in range(2):  # head within pair
                        p0 = sub * 64
                        psc = sc_psum.tile([P, S], F32, tag="sc")
                        for nn in range(2):
                            nc.tensor.matmul(
                                psc[:, nn * 512 : (nn + 1) * 512],
                                qT_sb[p0 : p0 + 64, hp, i * P : (i + 1) * P],
                                kT_sb[p0 : p0 + 64, hp, nn * 512 : (nn + 1) * 512],
                                start=True, stop=True,
                            )
                        ps.append(psc)
                    # rowwise min/max -> alpha/beta for fused sigmoid
                    # layout: [maxA, maxB, minA, minB, betaA, betaB, alpA, alpB]
                    st = sstat.tile([P, 8], F32, tag="st")
                    wraws = []
                    for sub in range(2):
                        # bf16: halves the GpSimd min level and the ACT
                        # sigmoid read; costs ~0.1% extra rel err (simulated)
                        wraw = wraw_pool.tile([P, S], BF, tag="wr")
                        # fused PSUM->SBUF copy + row-max (accum) on DVE
                        nc.vector.tensor_scalar(
                            out=wraw[:], in0=ps[sub][:],
                            scalar1=-3.0e38, scalar2=None,
                            op0=mybir.AluOpType.max, op1=mybir.AluOpType.max,
                            accum_out=st[:, sub : sub + 1],
                        )
                        wraws.append(wraw)
                        nc.vector.tensor_reduce(
                            st[:, 2 + sub : 3 + sub], wraw[:],
                            axis=mybir.AxisListType.X, op=mybir.AluOpType.min,
                        )
                    mx = st[:, 0:2]
                    mn = st[:, 2:4]
                    rng = st[:, 4:6]
                    alp = st[:, 6:8]
                    nc.vector.tensor_sub(rng, mx, mn)
                    nc.vector.reciprocal(rng, rng)
                    nc.vector.tensor_scalar_mul(alp, rng, 10.0)
                    # beta = -(mn*alpha) - 5  (reuse rng slot)
                    nc.vector.tensor_mul(rng, mn, alp)
                    nc.vector.tensor_scalar(
                        out=rng, in0=rng, scalar1=-1.0, scalar2=-5.0,
                        op0=mybir.AluOpType.mult, op1=mybir.AluOpType.add,
                    )
                    wts = []
                    for sub in range(2):
                        w_t = w_pool.tile([P, S], BF, tag="w")
                        nc.scalar.activation(
                            out=w_t[:], in_=wraws[sub][:],
                            func=mybir.ActivationFunctionType.Sigmoid,
                            bias=st[:, 4 + sub : 5 + sub],
                            scale=st[:, 6 + sub : 7 + sub],
                        )
                        wt_t = wt_pool.tile([P, NCH, P], BF, tag="wt")
                        # split the two ~1.3us transpose desc-gens across the
                        # Sync and ACT HWDGE sequencers (Sync saturates with
                        # both)
                        if sub == 0:
                            nc.sync.dma_start_transpose(wt_t[:], w_t[:])
                        else:
                            nc.scalar.dma_start_transpose(wt_t[:], w_t[:])
                        wts.append(wt_t)
                    pending.append((hp, i, wts))

                units = [(hp, i) for hp in range(4) for i in range(NCH)]
                # vhat work list: (vp, c) pieces.  Block vp (= V columns of
                # head vp) must be done before head pair vp//2's out-stages
                # start at unit 8*(vp//2)+PEND.  Spread evenly so PE stays
                # the binding engine in every unit, with the wvt DMA slices
                # streaming in behind the gather traffic.
                PEND = 5
                pieces = [(vp, c) for vp in range(8) for c in range(NCH)]
                # blocks 0-1 (16 pieces) across units 0-4; blocks 2-7 (48)
                # at 2/unit over units 5-28
                vhat_sched = {u: pieces[3 * u : 3 * u + 3] for u in range(4)}
                vhat_sched[4] = pieces[12:16]
                late = pieces[16:]
                for k in range(24):
                    vhat_sched[5 + k] = late[2 * k : 2 * k + 2]

                def vhat_piece(vp, c, eng=0):
                    # V columns of head vp, token chunk c.
                    lo = vp * 512
                    pv = vh_psum.tile([P, 512], F32, tag="pv")
                    for ec in range(EC):
                        nc.tensor.matmul(
                            pv[:],
                            hT_sb[:, ec, c * P : (c + 1) * P],
                            wvt_sb[:, ec, lo : lo + 512],
                            start=(ec == 0), stop=(ec == EC - 1),
                        )
                    # alternate the psum->SBUF copy between ACT and DVE to
                    # keep either engine from saturating
                    if eng == 0:
                        nc.scalar.copy(vh_sb[:, c, lo : lo + 512], pv[:])
                    else:
                        nc.vector.tensor_copy(vh_sb[:, c, lo : lo + 512], pv[:])

                def out_stage(args):
                    hp_, i_, wts = args
                    for sub in range(2):
                        h_idx = 2 * hp_ + sub
                        wt_t = wts[sub]
                        po = out_psum.tile([P, E], F32, tag="po")
                        for cc in range(NCH):
                            nc.tensor.matmul(
                                po[:],
                                wt_t[:, cc, :],
                                vh_sb[:, cc, h_idx * E : (h_idx + 1) * E],
                                start=(cc == 0), stop=(cc == NCH - 1),
                            )
                        ob = ob_pool.tile([P, E], BF, tag="ob")
                        nc.scalar.copy(ob[:], po[:])
                        # issue the store from the ACT sequencer: the Sync
                        # sequencer is near-saturated by the W^T transpose
                        # descriptor generation
                        nc.scalar.dma_start(
                            out_d[h_idx, i_ * P : (i_ + 1) * P, :], ob[:]
                        )

                for u, (hp, i) in enumerate(units):
                    if u in (2, 4, 6):
                        proj_slice(1, u // 2)
                    if u >= PEND:
                        out_stage(pending.pop(0))
                    for pi, (vp, c) in enumerate(vhat_sched.get(u, ())):
                        vhat_piece(vp, c, eng=pi % 2)
                    score_unit(hp, i)
                for pp_ in pending:
                    out_stage(pp_)

    nc.compile()
    _BUILD_CACHE[(use_beta,)] = nc
    return nc


def _prep_inputs(x, emb_table, gamma, beta, Wq, Wk, Wv, use_beta):
    x = np.asarray(x)
    gamma = np.asarray(gamma, dtype=np.float32)
    beta = np.asarray(beta, dtype=np.float32)
    Wq = np.asarray(Wq, dtype=np.float32)
    Wk = np.asarray(Wk, dtype=np.float32)
    Wv = np.asarray(Wv, dtype=np.float32)
    emb = np.ascontiguousarray(np.asarray(emb_table, dtype=np.float32))

    # W'[h,a,e] = W[h,a,e] * gamma[e]; layouts [e, h*ad+a]
    wqt = np.ascontiguousarray(
        (Wq * gamma[None, None, :]).reshape(H * A, E).T.astype(BF16)
    )
    wkt = np.ascontiguousarray(
        (Wk * gamma[None, None, :]).reshape(H * A, E).T.astype(BF16)
    )
    wvt = np.ascontiguousarray(
        (Wv * gamma[None, None, :]).reshape(H * E, E).T.astype(BF16)
    )
    idn = np.eye(P, dtype=np.float32).astype(BF16)

    consts = dict(emb=emb, wqt=wqt, wkt=wkt, wvt=wvt, idn=idn)
    if use_beta:
        qb = (Wq.reshape(H * A, E) @ beta).astype(np.float32)   # [512]
        kb = (Wk.reshape(H * A, E) @ beta).astype(np.float32)
        vb = (Wv.reshape(H * E, E) @ beta).astype(np.float32)   # [4096]
        consts["qb"] = np.ascontiguousarray(qb.reshape(4, P).T)
        consts["kb"] = np.ascontiguousarray(kb.reshape(4, P).T)
        consts["vb"] = vb.reshape(1, H * E)

    in_maps = []
    for b in range(B):
        xi = x[b].astype(np.int64)
        idx16 = np.ascontiguousarray(
            xi.reshape(S // 16, 16).T.astype(np.int16)
        )  # [16, 64]; token j of chunk c sits at [j%16, 8c + j//16]
        idx_full = np.ascontiguousarray(np.tile(idx16, (8, 1)))  # [128, 64]
        in_maps.append(dict(idx=idx_full, **consts))
    return in_maps


def kernel(x, emb_table, gamma, beta, Wq, Wk, Wv):
    global LAST_RESULTS
    beta_arr = np.asarray(beta, dtype=np.float32)
    use_beta = bool(np.any(beta_arr != 0.0))

    nc = build_nc(use_beta)
    in_maps = _prep_inputs(x, emb_table, gamma, beta, Wq, Wk, Wv, use_beta)

    trace = os.environ.get("KERNEL_TRACE", "0") == "1"
    res = run_bass_kernel_spmd(
        nc, in_maps, core_ids=list(range(B)), trace=trace
    )
    LAST_RESULTS = res

    out = np.stack([np.asarray(res.results[b]["out"]) for b in range(B)], axis=0)
    return out.astype(np.float32)


if __name__ == "__main__":
    rng = np.random.default_rng(0)
    x = rng.integers(0, VOCAB, size=(B, S), dtype=np.int32)
    emb = rng.standard_normal((VOCAB, E), dtype=np.float32)
    gamma = np.ones(E, np.float32)
    beta = np.zeros(E, np.float32)
    Wq = rng.random((H, A, E), dtype=np.float32)
    Wk = rng.random((H, A, E), dtype=np.float32)
    Wv = rng.random((H, E, E), dtype=np.float32)
    out = kernel(x, emb, gamma, beta, Wq, Wk, Wv)
    print(out.shape, out.dtype)
